# revision 17
# baseline (speedup 1.0000x reference)
"""BiMamba encoder on 8 trn2 NeuronCores — batch-sharded (4 seqs/core).

Layout: channel-major [channel, b*1024+t] activations in SBUF.
- PE: all projections, partition-broadcasts (ones-matmul), LN reductions
- ACT: exp(dt*A) via per-partition scale, silu/softplus, psum->sbuf copies
- DVE: tensor_tensor_scan for the selective-scan recurrence (full 1024-step
  sequences, zero initial state), elementwise mults/adds
- Reverse direction: negative-stride (reversed) APs — no data flipping.
- fwd+rev out_proj accumulate into one PSUM group with 0.5 folded into
  out_w host-side.
"""
import sys
import contextlib
import numpy as np

for _p in ("/opt/trn_rl_repo",):
    if _p not in sys.path:
        sys.path.insert(0, _p)

import concourse.bass as bass
import concourse.tile as tile
from concourse import mybir
from concourse.bass_utils import run_bass_kernel_spmd
from concourse.vector_clock import ScopedClock, VectorClock

# ---------------------------------------------------------------- dims
DM = 256        # d_model
DI = 512        # d_inner
NST = 16        # d_state
NL = 4          # layers
BLOC = 4        # sequences per core
L = 1024
COLS = BLOC * L  # 4096
NCORES = 8
F32 = mybir.dt.float32
BF16 = mybir.dt.bfloat16
AOP = mybir.AluOpType
AF = mybir.ActivationFunctionType

# ------------------------------------------------- tile drain workaround
# walrus codegen in this container rejects >1 sync-wait per instruction on
# the TileContext end-of-kernel Drain; hoist waits onto 1-wait SP NOPs.


def _patched_drain_and_barrier(self, tick_clock, wait_clock):
    nc = self.nc
    vc = tick_clock.global_clock
    n = len(vc)
    for i in range(n):
        if vc[i] > 0:
            sub = [0] * n
            sub[i] = vc[i]
            nop_inst = nc.sync.nop(nofuse=True, hint="drain_wait_split")
            wait_clock.add_sem_waits(
                nop_inst.ins, ScopedClock({None: VectorClock(sub)}))
    nc.sync.drain()
    nc.all_engine_barrier()
    assert self.sems is not None
    popped = nc._tile_sem_poison_stack.pop()
    assert popped is self._sem_poison
    nc.clear_and_free_semaphores(list(self.sems.allocated().values()))
    nc.all_engine_barrier()


tile.TileContext._drain_and_barrier = _patched_drain_and_barrier


# This container's walrus codegen rejects any instruction carrying more than
# one sync-wait. Post-process the serialized BIR: hoist extra waits onto
# same-engine NoOps inserted immediately before the instruction.
def _split_multi_waits_json(raw: bytes) -> bytes:
    import json
    d = json.loads(raw)
    changed = False
    for fn in d.get("functions", []):
        for blk in fn.get("blocks", []):
            insts = blk.get("instructions", [])
            out = []
            for inst in insts:
                si = inst.get("sync_info")
                waits = si.get("on_wait") if si else None
                if waits and len(waits) > 1:
                    changed = True
                    for k, wt in enumerate(waits[:-1]):
                        out.append({
                            "debug": inst.get("debug", 0),
                            "engine": inst["engine"],
                            "ins": [],
                            "name": f"ws{k}_{inst['name']}",
                            "opcode": "NoOp",
                            "outs": [],
                            "sync_info": {"on_update": [], "on_wait": [wt]},
                        })
                    si["on_wait"] = [waits[-1]]
                out.append(inst)
            blk["instructions"] = out
    if not changed:
        return raw
    return json.dumps(d).encode()


_orig_to_json_bytes = bass.Bass.to_json_bytes


def _patched_to_json_bytes(self, *a, **kw):
    return _split_multi_waits_json(_orig_to_json_bytes(self, *a, **kw))


bass.Bass.to_json_bytes = _patched_to_json_bytes


# ---------------------------------------------------------------- builder
def build_nc(n_layers=NL):
    nc = bass.Bass()
    P = {}

    def dparam(name, shape):
        P[name] = nc.declare_dram_parameter(name, list(shape), F32, isOutput=False)
        return P[name]

    dparam("data", (5, COLS))          # proto, plen, flags, iat, dir
    dparam("miscC", (128, 12))         # iota(2) lnWB(4) normWB(4) eps(1) ones(1)
    dparam("embP", (128, 64))          # rows 0:128 -> cols 0:32, rows 128:256 -> 32:64
    dparam("embF", (64, 32))
    dparam("dirD", (5, 8))             # row4 = emb_dir[1]-emb_dir[0]
    dparam("dirB", (8, 1))             # emb_dir[0]
    dparam("lenW", (5, 32))
    dparam("lenB", (32, 1))
    dparam("iatW", (5, 32))
    dparam("iatB", (32, 1))
    dparam("fusW1", (128, 256))        # fus_w.T rows 0:128
    dparam("fusW2", (8, 256))          # fus_w.T rows 128:136
    dparam("fusB", (128, 2))
    P["selBC"] = nc.declare_dram_parameter("selBC", [32, 4096], BF16, isOutput=False)
    dparam("sel5", (5, 256))           # fp32: block0 = row0, block1 = row2
    for li in range(n_layers):
        for d in ("f", "r"):
            pre = f"m{li}{d}_"
            dparam(pre + "inW", (256, 1024))   # in_w.T
            dparam(pre + "convW", (128, 16))   # [dblk*4+k] cols
            dparam(pre + "convB", (128, 8))
            dparam(pre + "xpW", (512, 48))     # xproj_w.T
            dparam(pre + "dtW", (16, 512))     # dt_w.T
            dparam(pre + "dtB", (128, 4))
            dparam(pre + "A", (128, 64))       # [dblk*16+n] cols
            dparam(pre + "D", (128, 4))
            dparam(pre + "outW", (512, 256))   # out_w.T * 0.5
    out_ext = nc.declare_dram_parameter("out", [DM, COLS], F32, isOutput=True)

    TCH = 512                 # matmul rhs chunk
    NCH = COLS // TCH         # 8 chunks over all columns

    with tile.TileContext(nc) as tc:
        ctx = contextlib.ExitStack()
        with ctx:
            cpool = ctx.enter_context(tc.tile_pool(name="consts", bufs=1))
            fpool = ctx.enter_context(tc.tile_pool(name="feat", bufs=1))
            tmp2 = ctx.enter_context(tc.tile_pool(name="tmp2", bufs=2))
            # PSUM: mmA 1 + mmB 1 + mmC 2 = 4 banks; bcB 2 + bcC 2 = 4 banks
            mmpA = ctx.enter_context(tc.tile_pool(name="mmA", bufs=1, space="PSUM"))
            mmpB = ctx.enter_context(tc.tile_pool(name="mmB", bufs=1, space="PSUM"))
            mmpC = ctx.enter_context(tc.tile_pool(name="mmC", bufs=2, space="PSUM"))
            bcp = ctx.enter_context(tc.tile_pool(name="bc", bufs=1, space="PSUM"))

            # ---------------- constants
            miscC = cpool.tile([128, 12], F32, tag="miscC")
            nc.sync.dma_start(out=miscC[:], in_=P["miscC"][:])
            iota = miscC[:, 0:2]
            lnWB = miscC[:, 2:6]
            normWB = miscC[:, 6:10]
            eps_c = miscC[:, 10:11]
            ones_col = miscC[:, 11:12]                             # colsum lhsT
            selBC = cpool.tile([32, 4096], BF16, tag="selBC")
            nc.sync.dma_start(out=selBC[:], in_=P["selBC"][:])
            sel5 = cpool.tile([5, 256], F32, tag="sel5")
            nc.sync.dma_start(out=sel5[:], in_=P["sel5"][:])
            ones_row = sel5[0:1, 0:128]                            # bcast lhsT

            # feat tiles (persistent residual stream)
            F = [fpool.tile([128, COLS], F32, tag=f"feat{i}", name=f"feat{i}")
                 for i in range(2)]

            # LN over channels for one 512-col chunk.
            # xap0/xap1: [128, TCH] APs (pre-LN input); dst0/dst1: output APs.
            # wb: [128, 4] tile with cols (w0, w1, b0, b1).
            def layer_norm_chunk(xap0, xap1, dst0, dst1, wb):
                s1 = mmpA.tile([1, TCH], F32, tag="lnS")
                nc.tensor.matmul(s1[:], ones_col[:], xap0, start=True, stop=False)
                nc.tensor.matmul(s1[:], ones_col[:], xap1, start=False, stop=True)
                sq0 = tmp2.tile([128, TCH], F32, tag="lnsq")
                nc.scalar.activation(sq0[:], xap0, AF.Square)
                sq1 = tmp2.tile([128, TCH], F32, tag="lnsq")
                nc.scalar.activation(sq1[:], xap1, AF.Square)
                s2 = mmpB.tile([1, TCH], F32, tag="lnS2")
                nc.tensor.matmul(s2[:], ones_col[:], sq0[:], start=True, stop=False)
                nc.tensor.matmul(s2[:], ones_col[:], sq1[:], start=False, stop=True)
                s1r = tmp2.tile([1, TCH], F32, tag="lnsq", name="s1r")
                nc.scalar.activation(s1r[:], s1[:], AF.Copy, scale=1.0 / DM)
                s2r = tmp2.tile([1, TCH], F32, tag="lnsq", name="s2r")
                nc.scalar.activation(s2r[:], s2[:], AF.Copy, scale=1.0 / DM)
                mu = mmpA.tile([128, TCH], F32, tag="lnS")
                nc.tensor.matmul(mu[:], ones_row[:], s1r[:], start=True, stop=True)
                e2 = mmpB.tile([128, TCH], F32, tag="lnS2")
                nc.tensor.matmul(e2[:], ones_row[:], s2r[:], start=True, stop=True)
                musq = tmp2.tile([128, TCH], F32, tag="lnsq")
                nc.scalar.activation(musq[:], mu[:], AF.Square)
                var = tmp2.tile([128, TCH], F32, tag="lnsq")
                nc.vector.tensor_tensor(out=var[:], in0=e2[:], in1=musq[:],
                                        op=AOP.subtract)
                lnv = tmp2.tile([128, TCH], F32, tag="lnsq")
                nc.scalar.activation(lnv[:], var[:], AF.Ln, bias=eps_c[:])
                rstd = tmp2.tile([128, TCH], F32, tag="lnsq")
                nc.scalar.activation(rstd[:], lnv[:], AF.Exp, scale=-0.5)
                for blk, (xap, dst) in enumerate(((xap0, dst0), (xap1, dst1))):
                    cen = tmp2.tile([128, TCH], F32, tag="lncen", bufs=1)
                    nc.vector.tensor_tensor(out=cen[:], in0=xap, in1=mu[:],
                                            op=AOP.subtract)
                    nc.vector.tensor_tensor(out=cen[:], in0=cen[:], in1=rstd[:],
                                            op=AOP.mult)
                    nc.scalar.activation(dst, cen[:], AF.Identity,
                                         scale=wb[:, blk:blk + 1],
                                         bias=wb[:, 2 + blk:3 + blk])

            # ================ tokenizer (scoped pool) ================
            with tc.tile_pool(name="tok", bufs=1) as tokp:
                data = tokp.tile([5, COLS], F32, tag="tokdata")
                nc.sync.dma_start(out=data[:], in_=P["data"][:])
                embP = tokp.tile([128, 64], F32, tag="embP")
                nc.sync.dma_start(out=embP[:], in_=P["embP"][:])
                embF = tokp.tile([64, 32], F32, tag="embF")
                nc.sync.dma_start(out=embF[:], in_=P["embF"][:])

                def tconst(name, shape, tag):
                    t = tokp.tile(list(shape), F32, tag=tag)
                    nc.sync.dma_start(out=t[:], in_=P[name][:])
                    return t

                dirD = tconst("dirD", (5, 8), "dirD")
                dirB = tconst("dirB", (8, 1), "dirB")
                lenW = tconst("lenW", (5, 32), "lenW")
                lenB = tconst("lenB", (32, 1), "lenB")
                iatW = tconst("iatW", (5, 32), "iatW")
                iatB = tconst("iatB", (32, 1), "iatB")
                fusW1 = tconst("fusW1", (128, 256), "fusW1")
                fusW2 = tconst("fusW2", (8, 256), "fusW2")
                fusB = tconst("fusB", (128, 2), "fusB")

                ohP0 = tokp.tile([128, COLS], F32, tag="ohP0")
                ohP1 = tokp.tile([128, COLS], F32, tag="ohP1")
                ohF = tokp.tile([64, COLS], F32, tag="ohF")
                c1 = tokp.tile([128, COLS], F32, tag="tokc1")
                c2 = tokp.tile([8, COLS], F32, tag="tokc2")
                for ch in range(NCH):
                    cs = slice(ch * TCH, (ch + 1) * TCH)
                    idxb = bcp.tile([128, TCH], F32, tag="bcB")
                    nc.tensor.matmul(idxb[:], sel5[:, 0:128], data[:, cs],
                                     start=True, stop=True)
                    nc.vector.tensor_scalar(out=ohP0[:, cs], in0=idxb[:],
                                            scalar1=iota[:, 0:1], scalar2=None,
                                            op0=AOP.is_equal)
                    nc.vector.tensor_scalar(out=ohP1[:, cs], in0=idxb[:],
                                            scalar1=iota[:, 1:2], scalar2=None,
                                            op0=AOP.is_equal)
                    idxf = bcp.tile([128, TCH], F32, tag="bcC")
                    nc.tensor.matmul(idxf[:], sel5[:, 128:256], data[:, cs],
                                     start=True, stop=True)
                    nc.vector.tensor_scalar(out=ohF[:, cs], in0=idxf[0:64, :],
                                            scalar1=iota[0:64, 0:1], scalar2=None,
                                            op0=AOP.is_equal)
                    pp = mmpA.tile([32, TCH], F32, tag="lnS")
                    nc.tensor.matmul(pp[:], embP[:, 0:32], ohP0[:, cs],
                                     start=True, stop=False)
                    nc.tensor.matmul(pp[:], embP[:, 32:64], ohP1[:, cs],
                                     start=False, stop=True)
                    nc.scalar.activation(c1[0:32, cs], pp[:], AF.Copy)
                    pl = mmpB.tile([32, TCH], F32, tag="lnS2")
                    nc.tensor.matmul(pl[:], lenW[:], data[:, cs],
                                     start=True, stop=True)
                    nc.scalar.activation(c1[32:64, cs], pl[:], AF.Identity,
                                         bias=lenB[:])
                    pf = mmpA.tile([32, TCH], F32, tag="lnS")
                    nc.tensor.matmul(pf[:], embF[:], ohF[:, cs],
                                     start=True, stop=True)
                    nc.scalar.activation(c1[64:96, cs], pf[:], AF.Copy)
                    pi = mmpB.tile([32, TCH], F32, tag="lnS2")
                    nc.tensor.matmul(pi[:], iatW[:], data[:, cs],
                                     start=True, stop=True)
                    nc.scalar.activation(c1[96:128, cs], pi[:], AF.Identity,
                                         bias=iatB[:])
                    pd = mmpA.tile([8, TCH], F32, tag="lnS")
                    nc.tensor.matmul(pd[:], dirD[:], data[:, cs],
                                     start=True, stop=True)
                    nc.scalar.activation(c2[:, cs], pd[:], AF.Identity, bias=dirB[:])
                # fus + LN -> feat
                for ch in range(NCH):
                    cs = slice(ch * TCH, (ch + 1) * TCH)
                    xln = []
                    for mblk in range(2):
                        ps = mmpC.tile([128, TCH], F32, tag="mmC")
                        nc.tensor.matmul(ps[:],
                                         fusW1[:, mblk * 128:(mblk + 1) * 128],
                                         rhs=c1[:, cs], start=True, stop=False)
                        nc.tensor.matmul(ps[:],
                                         fusW2[:, mblk * 128:(mblk + 1) * 128],
                                         rhs=c2[:, cs], start=False, stop=True)
                        xt = tmp2.tile([128, TCH], F32, tag="lnx")
                        nc.scalar.activation(xt[:], ps[:], AF.Identity,
                                             bias=fusB[:, mblk:mblk + 1])
                        xln.append(xt)
                    layer_norm_chunk(xln[0][:], xln[1][:],
                                     F[0][:, cs], F[1][:, cs], lnWB)

            # ================ mamba layers ================
            wpool2 = ctx.enter_context(tc.tile_pool(name="wbig", bufs=1))
            wpool1 = ctx.enter_context(tc.tile_pool(name="wsmall", bufs=1))
            apool = ctx.enter_context(tc.tile_pool(name="acts", bufs=1))
            spA = ctx.enter_context(tc.tile_pool(name="spA", bufs=2))
            spB = ctx.enter_context(tc.tile_pool(name="spB", bufs=1))
            spH = ctx.enter_context(tc.tile_pool(name="spH", bufs=1))

            for li in range(n_layers):
                W = {}
                for d in ("f", "r"):
                    pre = f"m{li}{d}_"
                    inW = wpool2.tile([128, 2048], F32, tag="inW" + d)
                    nc.sync.dma_start(out=inW[:, 0:1024],
                                      in_=P[pre + "inW"][0:128, :])
                    nc.sync.dma_start(out=inW[:, 1024:2048],
                                      in_=P[pre + "inW"][128:256, :])
                    outW = wpool1.tile([128, 1024], F32, tag="outW" + d)
                    for k in range(4):
                        nc.sync.dma_start(
                            out=outW[:, k * 256:(k + 1) * 256],
                            in_=P[pre + "outW"][k * 128:(k + 1) * 128, :])
                    xpW = wpool1.tile([128, 192], F32, tag="xpW" + d)
                    for k in range(4):
                        nc.sync.dma_start(
                            out=xpW[:, k * 48:(k + 1) * 48],
                            in_=P[pre + "xpW"][k * 128:(k + 1) * 128, :])
                    dtW = wpool1.tile([16, 512], F32, tag="dtW" + d)
                    nc.sync.dma_start(out=dtW[:], in_=P[pre + "dtW"][:])
                    Asb = wpool1.tile([128, 64], F32, tag="A" + d)
                    nc.sync.dma_start(out=Asb[:], in_=P[pre + "A"][:])
                    convW = wpool1.tile([128, 16], F32, tag="convW" + d)
                    nc.sync.dma_start(out=convW[:], in_=P[pre + "convW"][:])
                    convB = wpool1.tile([128, 8], F32, tag="convB" + d)
                    nc.sync.dma_start(out=convB[:], in_=P[pre + "convB"][:])
                    dtB = wpool1.tile([128, 4], F32, tag="dtB" + d)
                    nc.sync.dma_start(out=dtB[:], in_=P[pre + "dtB"][:])
                    Dsb = wpool1.tile([128, 4], F32, tag="D" + d)
                    nc.sync.dma_start(out=Dsb[:], in_=P[pre + "D"][:])
                    W[d] = dict(inW=inW, outW=outW, xpW=xpW, dtW=dtW, A=Asb,
                                convW=convW, convB=convB, dtB=dtB, D=Dsb)

                for b in range(BLOC):
                    base = b * L
                    y2 = {}
                    for d in ("f", "r"):
                        rev = (d == "r")
                        Wd = W[d]
                        # ---- in_proj -> xi (raw), zs (silu(z))
                        xi = [apool.tile([128, L], F32, tag=f"xi{k}", name=f"xi{k}")
                              for k in range(4)]
                        zs = [apool.tile([128, L], BF16, tag=f"zs{k}", name=f"zs{k}")
                              for k in range(4)]
                        for m in range(8):
                            for t2 in range(2):
                                cs = slice(base + t2 * TCH, base + (t2 + 1) * TCH)
                                ls = slice(t2 * TCH, (t2 + 1) * TCH)
                                ps = mmpC.tile([128, TCH], F32, tag="mmC")
                                nc.tensor.matmul(
                                    ps[:], Wd["inW"][:, m * 128:(m + 1) * 128],
                                    rhs=F[0][:, cs], start=True, stop=False)
                                nc.tensor.matmul(
                                    ps[:],
                                    Wd["inW"][:, 1024 + m * 128:1024 + (m + 1) * 128],
                                    rhs=F[1][:, cs], start=False, stop=True)
                                if m < 4:
                                    nc.scalar.activation(xi[m][:, ls], ps[:],
                                                         AF.Copy)
                                else:
                                    e_z = tmp2.tile([128, TCH], F32, tag="lnsq",
                                                    name="e_z")
                                    nc.scalar.activation(e_z[:], ps[:], AF.Exp,
                                                         scale=-1.0)
                                    sp_z = tmp2.tile([128, TCH], F32, tag="lnsq",
                                                     name="sp_z")
                                    nc.scalar.activation(sp_z[:], e_z[:], AF.Ln,
                                                         bias=1.0)
                                    sg_z = tmp2.tile([128, TCH], F32, tag="lnsq",
                                                     name="sg_z")
                                    nc.scalar.activation(sg_z[:], sp_z[:], AF.Exp,
                                                         scale=-1.0)
                                    nc.vector.tensor_tensor(
                                        out=zs[m - 4][:, ls], in0=ps[:],
                                        in1=sg_z[:], op=AOP.mult)
                        # ---- depthwise conv (causal fwd / anti-causal rev) + silu
                        xc = [apool.tile([128, L], F32, tag=f"xc{k}", name=f"xc{k}")
                              for k in range(4)]
                        for k4 in range(4):
                            cw = Wd["convW"]
                            acc = tmp2.tile([128, L], F32, tag="convacc")
                            nc.vector.tensor_scalar(
                                out=acc[:], in0=xi[k4][:],
                                scalar1=cw[:, k4 * 4 + 3:k4 * 4 + 4], scalar2=None,
                                op0=AOP.mult)
                            for k in range(3):
                                sh = 3 - k
                                if not rev:
                                    oap = acc[:, sh:L]
                                    iap = xi[k4][:, 0:L - sh]
                                else:
                                    oap = acc[:, 0:L - sh]
                                    iap = xi[k4][:, sh:L]
                                nc.vector.scalar_tensor_tensor(
                                    out=oap, in0=iap,
                                    scalar=cw[:, k4 * 4 + k:k4 * 4 + k + 1],
                                    in1=oap, op0=AOP.mult, op1=AOP.add)
                            e_x = tmp2.tile([128, L], F32, tag="sigx", name="e_x")
                            nc.scalar.activation(
                                e_x[:], acc[:], AF.Exp, scale=-1.0,
                                bias=Wd["convB"][:, 4 + k4:5 + k4])
                            sp_x = tmp2.tile([128, L], F32, tag="sigx", name="sp_x")
                            nc.scalar.activation(sp_x[:], e_x[:], AF.Ln, bias=1.0)
                            sg_x = tmp2.tile([128, L], F32, tag="sigx", name="sg_x")
                            nc.scalar.activation(sg_x[:], sp_x[:], AF.Exp,
                                                 scale=-1.0)
                            nc.vector.scalar_tensor_tensor(
                                out=xc[k4][:], in0=acc[:],
                                scalar=Wd["convB"][:, k4:k4 + 1], in1=sg_x[:],
                                op0=AOP.add, op1=AOP.mult)
                        # ---- xproj -> dtin [16, L], bc [32, L] (B | C)
                        dtin = tmp2.tile([16, L], F32, tag="convacc",
                                          name="dtin")
                        bc = apool.tile([32, L], BF16, tag="bcs")
                        for t2 in range(2):
                            ls = slice(t2 * TCH, (t2 + 1) * TCH)
                            ps = mmpC.tile([16, TCH], F32, tag="mmC")
                            for k in range(4):
                                nc.tensor.matmul(
                                    ps[:],
                                    Wd["xpW"][:, k * 48:k * 48 + 16],
                                    rhs=xc[k][:, ls], start=(k == 0),
                                    stop=(k == 3))
                            nc.scalar.activation(dtin[:, ls], ps[:], AF.Copy)
                            ps2 = mmpC.tile([32, TCH], F32, tag="mmC")
                            for k in range(4):
                                nc.tensor.matmul(
                                    ps2[:],
                                    Wd["xpW"][:, k * 48 + 16:(k + 1) * 48],
                                    rhs=xc[k][:, ls], start=(k == 0),
                                    stop=(k == 3))
                            nc.scalar.activation(bc[:, ls], ps2[:], AF.Copy)
                        # ---- dt = softplus(dt_w @ dt_in + dt_b)
                        dt = [apool.tile([128, L], F32, tag=f"dt{k}", name=f"dt{k}")
                              for k in range(4)]
                        for k4 in range(4):
                            for t2 in range(2):
                                ls = slice(t2 * TCH, (t2 + 1) * TCH)
                                ps = mmpC.tile([128, TCH], F32, tag="mmC")
                                nc.tensor.matmul(
                                    ps[:], Wd["dtW"][:, k4 * 128:(k4 + 1) * 128],
                                    rhs=dtin[:, ls], start=True, stop=True)
                                e_d = tmp2.tile([128, TCH], F32, tag="lnsq",
                                                name="e_d")
                                nc.scalar.activation(e_d[:], ps[:], AF.Exp,
                                                     bias=Wd["dtB"][:, k4:k4 + 1])
                                nc.scalar.activation(dt[k4][:, ls], e_d[:], AF.Ln,
                                                     bias=1.0)
                        # ---- w = dt * xc
                        wti = [apool.tile([128, L], BF16, tag=f"w{k}", name=f"w{k}")
                               for k in range(4)]
                        for k4 in range(4):
                            nc.vector.tensor_tensor(out=wti[k4][:], in0=dt[k4][:],
                                                    in1=xc[k4][:], op=AOP.mult)
                        # ---- state scan over n, accumulate y
                        yacc = [apool.tile([128, L], F32, tag=f"ya{k}",
                                            name=f"ya{k}", bufs=2)
                                for k in range(4)]
                        for n in range(NST):
                            Bb = bcp.tile([128, L], F32, tag="bcB")
                            Cb = bcp.tile([128, L], F32, tag="bcC")
                            for t2 in range(2):
                                ls = slice(t2 * TCH, (t2 + 1) * TCH)
                                nc.tensor.matmul(
                                    Bb[:, ls], selBC[:, n * 128:(n + 1) * 128],
                                    rhs=bc[:, ls], start=True, stop=True)
                                nc.tensor.matmul(
                                    Cb[:, ls],
                                    selBC[:, (16 + n) * 128:(17 + n) * 128],
                                    rhs=bc[:, ls], start=True, stop=True)
                            for k4 in range(4):
                                dA = spA.tile([128, L], F32, tag="dA")
                                din = dt[k4][:, ::-1] if rev else dt[k4][:]
                                nc.scalar.activation(
                                    dA[:], din, AF.Exp,
                                    scale=Wd["A"][:, k4 * 16 + n:k4 * 16 + n + 1])
                                dB = spB.tile([128, L], BF16, tag="dB")
                                if rev:
                                    nc.vector.tensor_tensor(
                                        out=dB[:], in0=wti[k4][:, ::-1],
                                        in1=Bb[:, ::-1], op=AOP.mult)
                                else:
                                    nc.vector.tensor_tensor(
                                        out=dB[:], in0=wti[k4][:], in1=Bb[:],
                                        op=AOP.mult)
                                h = spH.tile([128, L], BF16, tag="h")
                                nc.vector.tensor_tensor_scan(
                                    out=h[:], data0=dA[:], data1=dB[:],
                                    initial=0.0, op0=AOP.mult, op1=AOP.add)
                                hap = h[:, ::-1] if rev else h[:]
                                if n == 0:
                                    nc.vector.tensor_tensor(
                                        out=yacc[k4][:], in0=hap, in1=Cb[:],
                                        op=AOP.mult)
                                else:
                                    p = spA.tile([128, L], F32, tag="dA")
                                    nc.vector.tensor_tensor(
                                        out=p[:], in0=hap, in1=Cb[:], op=AOP.mult)
                                    nc.vector.tensor_tensor(
                                        out=yacc[k4][:], in0=yacc[k4][:],
                                        in1=p[:], op=AOP.add)
                        # ---- tail: y2 = (yacc + xc*D) * silu(z)
                        for k4 in range(4):
                            nc.vector.scalar_tensor_tensor(
                                out=xc[k4][:], in0=xc[k4][:],
                                scalar=Wd["D"][:, k4:k4 + 1], in1=yacc[k4][:],
                                op0=AOP.mult, op1=AOP.add)
                            nc.vector.tensor_tensor(out=yacc[k4][:],
                                                    in0=xc[k4][:],
                                                    in1=zs[k4][:], op=AOP.mult)
                        y2[d] = yacc
                    # ---- fused out_proj (0.5 in weights) + residual + LN
                    for t2 in range(2):
                        cs = slice(base + t2 * TCH, base + (t2 + 1) * TCH)
                        ls = slice(t2 * TCH, (t2 + 1) * TCH)
                        xln = []
                        for mblk in range(2):
                            ps = mmpC.tile([128, TCH], F32, tag="mmC")
                            first = True
                            for d in ("f", "r"):
                                for k in range(4):
                                    nc.tensor.matmul(
                                        ps[:],
                                        W[d]["outW"][:, k * 256 + mblk * 128:
                                                     k * 256 + (mblk + 1) * 128],
                                        rhs=y2[d][k][:, ls], start=first,
                                        stop=(d == "r" and k == 3))
                                    first = False
                            xt = tmp2.tile([128, TCH], F32, tag="lnx")
                            nc.vector.tensor_tensor(out=xt[:], in0=ps[:],
                                                    in1=F[mblk][:, cs],
                                                    op=AOP.add)
                            xln.append(xt)
                        layer_norm_chunk(xln[0][:], xln[1][:],
                                         F[0][:, cs], F[1][:, cs], normWB)

            # ---------------- output
            nc.sync.dma_start(out=out_ext[0:128, :], in_=F[0][:])
            nc.sync.dma_start(out=out_ext[128:256, :], in_=F[1][:])
    return nc


# ---------------------------------------------------------------- host prep
def _blkfold(v):
    """[512] -> [128, 4] with col = dblk"""
    return np.ascontiguousarray(np.asarray(v, np.float32).reshape(4, 128).T)


def _pack_weights(tok, mambas_fwd, mambas_rev, norm_w, norm_b, n_layers=NL):
    w = {}
    misc = np.zeros((128, 12), np.float32)
    misc[:, 0] = np.arange(128, dtype=np.float32)
    misc[:, 1] = np.arange(128, 256, dtype=np.float32)
    misc[:, 2] = np.asarray(tok["ln_w"], np.float32)[0:128]
    misc[:, 3] = np.asarray(tok["ln_w"], np.float32)[128:256]
    misc[:, 4] = np.asarray(tok["ln_b"], np.float32)[0:128]
    misc[:, 5] = np.asarray(tok["ln_b"], np.float32)[128:256]
    misc[:, 6] = np.asarray(norm_w, np.float32)[0:128]
    misc[:, 7] = np.asarray(norm_w, np.float32)[128:256]
    misc[:, 8] = np.asarray(norm_b, np.float32)[0:128]
    misc[:, 9] = np.asarray(norm_b, np.float32)[128:256]
    misc[:, 10] = 1e-5
    misc[:, 11] = 1.0
    w["miscC"] = misc
    ep = np.asarray(tok["emb_proto"], np.float32)      # [256, 32]
    w["embP"] = np.ascontiguousarray(np.concatenate([ep[:128], ep[128:]], axis=1))
    w["embF"] = np.asarray(tok["emb_flags"], np.float32)
    ed = np.asarray(tok["emb_dir"], np.float32)
    dd = np.zeros((5, 8), np.float32); dd[4] = ed[1] - ed[0]
    w["dirD"] = dd
    w["dirB"] = np.ascontiguousarray(ed[0].reshape(8, 1))
    lw = np.zeros((5, 32), np.float32); lw[1] = np.asarray(tok["len_w"], np.float32)[:, 0]
    w["lenW"] = lw
    w["lenB"] = np.ascontiguousarray(np.asarray(tok["len_b"], np.float32).reshape(32, 1))
    iw = np.zeros((5, 32), np.float32); iw[3] = np.asarray(tok["iat_w"], np.float32)[:, 0]
    w["iatW"] = iw
    w["iatB"] = np.ascontiguousarray(np.asarray(tok["iat_b"], np.float32).reshape(32, 1))
    fw = np.asarray(tok["fus_w"], np.float32).T        # [136, 256]
    w["fusW1"] = np.ascontiguousarray(fw[0:128])
    w["fusW2"] = np.ascontiguousarray(fw[128:136])
    w["fusB"] = np.ascontiguousarray(
        np.asarray(tok["fus_b"], np.float32).reshape(2, 128).T)

    import ml_dtypes
    sel = np.zeros((32, 32 * 128), np.float32)
    for j in range(32):
        sel[j, j * 128:(j + 1) * 128] = 1.0
    w["selBC"] = sel.astype(ml_dtypes.bfloat16)
    s5 = np.zeros((5, 256), np.float32)
    s5[0, 0:128] = 1.0
    s5[2, 128:256] = 1.0
    w["sel5"] = s5

    for li in range(n_layers):
        for d, p in (("f", mambas_fwd[li]), ("r", mambas_rev[li])):
            pre = f"m{li}{d}_"
            w[pre + "inW"] = np.ascontiguousarray(np.asarray(p["in_w"], np.float32).T)
            cw = np.asarray(p["conv_w"], np.float32).reshape(512, 4)
            w[pre + "convW"] = np.ascontiguousarray(
                cw.reshape(4, 128, 4).transpose(1, 0, 2).reshape(128, 16))
            cb = _blkfold(p["conv_b"])
            w[pre + "convB"] = np.ascontiguousarray(
                np.concatenate([cb, -cb], axis=1))
            w[pre + "xpW"] = np.ascontiguousarray(
                np.asarray(p["xproj_w"], np.float32).T)
            w[pre + "dtW"] = np.ascontiguousarray(np.asarray(p["dt_w"], np.float32).T)
            w[pre + "dtB"] = _blkfold(p["dt_b"])
            A = -np.exp(np.asarray(p["A_log"], np.float32))  # [512, 16]
            w[pre + "A"] = np.ascontiguousarray(
                A.reshape(4, 128, 16).transpose(1, 0, 2).reshape(128, 64))
            w[pre + "D"] = _blkfold(p["D"])
            w[pre + "outW"] = np.ascontiguousarray(
                np.asarray(p["out_w"], np.float32).T * 0.5)
    return w


def _core_data(x, c):
    xb = np.asarray(x, np.float32)[c * BLOC:(c + 1) * BLOC]  # [4, 1024, 5]
    flat = xb.reshape(COLS, 5)
    d = np.empty((5, COLS), np.float32)
    d[0] = np.clip(flat[:, 0], 0, 255)
    d[1] = flat[:, 1]
    d[2] = np.clip(flat[:, 2], 0, 63)
    d[3] = flat[:, 3]
    d[4] = np.clip(flat[:, 4], 0, 1)
    return d


_NC_CACHE = {}


def get_nc(n_layers=NL):
    if n_layers not in _NC_CACHE:
        _NC_CACHE[n_layers] = build_nc(n_layers)
    return _NC_CACHE[n_layers]


def kernel(x, tok, mambas_fwd, mambas_rev, norm_w, norm_b):
    n_layers = len(mambas_fwd)
    nc = get_nc(n_layers)
    w = _pack_weights(tok, mambas_fwd, mambas_rev, norm_w, norm_b, n_layers)
    in_maps = []
    for c in range(NCORES):
        m = dict(w)
        m["data"] = _core_data(x, c)
        in_maps.append(m)
    res = run_bass_kernel_spmd(nc, in_maps, core_ids=list(range(NCORES)))
    outs = []
    for c in range(NCORES):
        o = res.results[c]["out"]                      # [256, 4096]
        outs.append(o.reshape(DM, BLOC, L).transpose(1, 2, 0))
    return np.concatenate(outs, axis=0).astype(np.float32)


# revision 19
# speedup vs baseline: 1.1230x; 1.1230x over previous
"""BiMamba encoder on 8 trn2 NeuronCores — batch-sharded (4 seqs/core).

Layout: channel-major [channel, b*1024+t] activations in SBUF.
- PE: all projections, partition-broadcasts (ones-matmul), LN reductions
- ACT: exp(dt*A) via per-partition scale, silu/softplus, psum->sbuf copies
- DVE: tensor_tensor_scan for the selective-scan recurrence (full 1024-step
  sequences, zero initial state), elementwise mults/adds
- Reverse direction: negative-stride (reversed) APs — no data flipping.
- fwd+rev out_proj accumulate into one PSUM group with 0.5 folded into
  out_w host-side.
"""
import sys
import contextlib
import numpy as np

for _p in ("/opt/trn_rl_repo",):
    if _p not in sys.path:
        sys.path.insert(0, _p)

import concourse.bass as bass
import concourse.tile as tile
from concourse import mybir
from concourse.bass_utils import run_bass_kernel_spmd
from concourse.vector_clock import ScopedClock, VectorClock

# ---------------------------------------------------------------- dims
DM = 256        # d_model
DI = 512        # d_inner
NST = 16        # d_state
NL = 4          # layers
BLOC = 4        # sequences per core
L = 1024
COLS = BLOC * L  # 4096
NCORES = 8
F32 = mybir.dt.float32
BF16 = mybir.dt.bfloat16
AOP = mybir.AluOpType
AF = mybir.ActivationFunctionType

# ------------------------------------------------- tile drain workaround
# walrus codegen in this container rejects >1 sync-wait per instruction on
# the TileContext end-of-kernel Drain; hoist waits onto 1-wait SP NOPs.


def _patched_drain_and_barrier(self, tick_clock, wait_clock):
    nc = self.nc
    vc = tick_clock.global_clock
    n = len(vc)
    for i in range(n):
        if vc[i] > 0:
            sub = [0] * n
            sub[i] = vc[i]
            nop_inst = nc.sync.nop(nofuse=True, hint="drain_wait_split")
            wait_clock.add_sem_waits(
                nop_inst.ins, ScopedClock({None: VectorClock(sub)}))
    nc.sync.drain()
    nc.all_engine_barrier()
    assert self.sems is not None
    popped = nc._tile_sem_poison_stack.pop()
    assert popped is self._sem_poison
    nc.clear_and_free_semaphores(list(self.sems.allocated().values()))
    nc.all_engine_barrier()


tile.TileContext._drain_and_barrier = _patched_drain_and_barrier


# This container's walrus codegen rejects any instruction carrying more than
# one sync-wait. Post-process the serialized BIR: hoist extra waits onto
# same-engine NoOps inserted immediately before the instruction.
def _split_multi_waits_json(raw: bytes) -> bytes:
    import json
    d = json.loads(raw)
    changed = False
    for fn in d.get("functions", []):
        for blk in fn.get("blocks", []):
            insts = blk.get("instructions", [])
            out = []
            for inst in insts:
                si = inst.get("sync_info")
                waits = si.get("on_wait") if si else None
                if waits and len(waits) > 1:
                    changed = True
                    for k, wt in enumerate(waits[:-1]):
                        out.append({
                            "debug": inst.get("debug", 0),
                            "engine": inst["engine"],
                            "ins": [],
                            "name": f"ws{k}_{inst['name']}",
                            "opcode": "NoOp",
                            "outs": [],
                            "sync_info": {"on_update": [], "on_wait": [wt]},
                        })
                    si["on_wait"] = [waits[-1]]
                out.append(inst)
            blk["instructions"] = out
    if not changed:
        return raw
    return json.dumps(d).encode()


_orig_to_json_bytes = bass.Bass.to_json_bytes


def _patched_to_json_bytes(self, *a, **kw):
    return _split_multi_waits_json(_orig_to_json_bytes(self, *a, **kw))


bass.Bass.to_json_bytes = _patched_to_json_bytes


# ---------------------------------------------------------------- builder
def build_nc(n_layers=NL):
    nc = bass.Bass()
    P = {}

    def dparam(name, shape):
        P[name] = nc.declare_dram_parameter(name, list(shape), F32, isOutput=False)
        return P[name]

    dparam("data", (5, COLS))          # proto, plen, flags, iat, dir
    dparam("miscC", (128, 12))         # iota(2) lnWB(4) normWB(4) eps(1) ones(1)
    dparam("embP", (128, 64))          # rows 0:128 -> cols 0:32, rows 128:256 -> 32:64
    dparam("embF", (64, 32))
    dparam("dirD", (5, 8))             # row4 = emb_dir[1]-emb_dir[0]
    dparam("dirB", (8, 1))             # emb_dir[0]
    dparam("lenW", (5, 32))
    dparam("lenB", (32, 1))
    dparam("iatW", (5, 32))
    dparam("iatB", (32, 1))
    dparam("fusW1", (128, 256))        # fus_w.T rows 0:128
    dparam("fusW2", (8, 256))          # fus_w.T rows 128:136
    dparam("fusB", (128, 2))
    P["selBC"] = nc.declare_dram_parameter("selBC", [32, 4096], BF16, isOutput=False)
    dparam("sel5", (5, 256))           # fp32: block0 = row0, block1 = row2
    for li in range(n_layers):
        for d in ("f", "r"):
            pre = f"m{li}{d}_"
            dparam(pre + "inW", (256, 1024))   # in_w.T
            dparam(pre + "convW", (128, 16))   # [dblk*4+k] cols
            dparam(pre + "convB", (128, 8))
            P[pre + "xpW"] = nc.declare_dram_parameter(pre + "xpW", [512, 48], BF16, isOutput=False)
            dparam(pre + "dtW", (16, 512))     # dt_w.T
            dparam(pre + "dtB", (128, 4))
            dparam(pre + "A", (128, 64))       # [dblk*16+n] cols
            dparam(pre + "D", (128, 4))
            dparam(pre + "outW", (512, 256))   # out_w.T * 0.5
    out_ext = nc.declare_dram_parameter("out", [DM, COLS], F32, isOutput=True)

    TCH = 512                 # matmul rhs chunk
    NCH = COLS // TCH         # 8 chunks over all columns

    with tile.TileContext(nc) as tc:
        ctx = contextlib.ExitStack()
        with ctx:
            cpool = ctx.enter_context(tc.tile_pool(name="consts", bufs=1))
            fpool = ctx.enter_context(tc.tile_pool(name="feat", bufs=1))
            tmp2 = ctx.enter_context(tc.tile_pool(name="tmp2", bufs=2))
            # PSUM: mmA 1 + mmB 1 + mmC 2 = 4 banks; bcB 2 + bcC 2 = 4 banks
            mmpA = ctx.enter_context(tc.tile_pool(name="mmA", bufs=1, space="PSUM"))
            mmpB = ctx.enter_context(tc.tile_pool(name="mmB", bufs=1, space="PSUM"))
            mmpC = ctx.enter_context(tc.tile_pool(name="mmC", bufs=2, space="PSUM"))
            bcp = ctx.enter_context(tc.tile_pool(name="bc", bufs=1, space="PSUM"))

            # ---------------- constants
            miscC = cpool.tile([128, 12], F32, tag="miscC")
            nc.sync.dma_start(out=miscC[:], in_=P["miscC"][:])
            iota = miscC[:, 0:2]
            lnWB = miscC[:, 2:6]
            normWB = miscC[:, 6:10]
            eps_c = miscC[:, 10:11]
            ones_col = miscC[:, 11:12]                             # colsum lhsT
            selBC = cpool.tile([32, 4096], BF16, tag="selBC")
            nc.sync.dma_start(out=selBC[:], in_=P["selBC"][:])
            sel5 = cpool.tile([5, 256], F32, tag="sel5")
            nc.sync.dma_start(out=sel5[:], in_=P["sel5"][:])
            ones_row = sel5[0:1, 0:128]                            # bcast lhsT

            # feat tiles (persistent residual stream)
            F = [fpool.tile([128, COLS], F32, tag=f"feat{i}", name=f"feat{i}")
                 for i in range(2)]

            # LN over channels for one 512-col chunk.
            # xap0/xap1: [128, TCH] APs (pre-LN input); dst0/dst1: output APs.
            # wb: [128, 4] tile with cols (w0, w1, b0, b1).
            def layer_norm_chunk(xap0, xap1, dst0, dst1, wb):
                s1 = mmpA.tile([1, TCH], F32, tag="lnS")
                nc.tensor.matmul(s1[:], ones_col[:], xap0, start=True, stop=False)
                nc.tensor.matmul(s1[:], ones_col[:], xap1, start=False, stop=True)
                sq0 = tmp2.tile([128, TCH], F32, tag="lnsq")
                nc.scalar.activation(sq0[:], xap0, AF.Square)
                sq1 = tmp2.tile([128, TCH], F32, tag="lnsq")
                nc.scalar.activation(sq1[:], xap1, AF.Square)
                s2 = mmpB.tile([1, TCH], F32, tag="lnS2")
                nc.tensor.matmul(s2[:], ones_col[:], sq0[:], start=True, stop=False)
                nc.tensor.matmul(s2[:], ones_col[:], sq1[:], start=False, stop=True)
                s1r = tmp2.tile([1, TCH], F32, tag="lnsq", name="s1r")
                nc.scalar.activation(s1r[:], s1[:], AF.Copy, scale=1.0 / DM)
                s2r = tmp2.tile([1, TCH], F32, tag="lnsq", name="s2r")
                nc.scalar.activation(s2r[:], s2[:], AF.Copy, scale=1.0 / DM)
                mu = mmpA.tile([128, TCH], F32, tag="lnS")
                nc.tensor.matmul(mu[:], ones_row[:], s1r[:], start=True, stop=True)
                e2 = mmpB.tile([128, TCH], F32, tag="lnS2")
                nc.tensor.matmul(e2[:], ones_row[:], s2r[:], start=True, stop=True)
                musq = tmp2.tile([128, TCH], F32, tag="lnsq")
                nc.scalar.activation(musq[:], mu[:], AF.Square)
                var = tmp2.tile([128, TCH], F32, tag="lnsq")
                nc.vector.tensor_tensor(out=var[:], in0=e2[:], in1=musq[:],
                                        op=AOP.subtract)
                lnv = tmp2.tile([128, TCH], F32, tag="lnsq")
                nc.scalar.activation(lnv[:], var[:], AF.Ln, bias=eps_c[:])
                rstd = tmp2.tile([128, TCH], F32, tag="lnsq")
                nc.scalar.activation(rstd[:], lnv[:], AF.Exp, scale=-0.5)
                for blk, (xap, dst) in enumerate(((xap0, dst0), (xap1, dst1))):
                    cen = tmp2.tile([128, TCH], F32, tag="lncen", bufs=1)
                    nc.vector.tensor_tensor(out=cen[:], in0=xap, in1=mu[:],
                                            op=AOP.subtract)
                    nc.vector.tensor_tensor(out=cen[:], in0=cen[:], in1=rstd[:],
                                            op=AOP.mult)
                    nc.scalar.activation(dst, cen[:], AF.Identity,
                                         scale=wb[:, blk:blk + 1],
                                         bias=wb[:, 2 + blk:3 + blk])

            # ================ tokenizer (scoped pool) ================
            with tc.tile_pool(name="tok", bufs=1) as tokp:
                data = tokp.tile([5, COLS], F32, tag="tokdata")
                nc.sync.dma_start(out=data[:], in_=P["data"][:])
                embP = tokp.tile([128, 64], F32, tag="embP")
                nc.sync.dma_start(out=embP[:], in_=P["embP"][:])
                embF = tokp.tile([64, 32], F32, tag="embF")
                nc.sync.dma_start(out=embF[:], in_=P["embF"][:])

                def tconst(name, shape, tag):
                    t = tokp.tile(list(shape), F32, tag=tag)
                    nc.sync.dma_start(out=t[:], in_=P[name][:])
                    return t

                dirD = tconst("dirD", (5, 8), "dirD")
                dirB = tconst("dirB", (8, 1), "dirB")
                lenW = tconst("lenW", (5, 32), "lenW")
                lenB = tconst("lenB", (32, 1), "lenB")
                iatW = tconst("iatW", (5, 32), "iatW")
                iatB = tconst("iatB", (32, 1), "iatB")
                fusW1 = tconst("fusW1", (128, 256), "fusW1")
                fusW2 = tconst("fusW2", (8, 256), "fusW2")
                fusB = tconst("fusB", (128, 2), "fusB")

                ohP0 = tokp.tile([128, COLS], F32, tag="ohP0")
                ohP1 = tokp.tile([128, COLS], F32, tag="ohP1")
                ohF = tokp.tile([64, COLS], F32, tag="ohF")
                c1 = tokp.tile([128, COLS], F32, tag="tokc1")
                c2 = tokp.tile([8, COLS], F32, tag="tokc2")
                for ch in range(NCH):
                    cs = slice(ch * TCH, (ch + 1) * TCH)
                    idxb = bcp.tile([128, TCH], F32, tag="bcB")
                    nc.tensor.matmul(idxb[:], sel5[:, 0:128], data[:, cs],
                                     start=True, stop=True)
                    nc.vector.tensor_scalar(out=ohP0[:, cs], in0=idxb[:],
                                            scalar1=iota[:, 0:1], scalar2=None,
                                            op0=AOP.is_equal)
                    nc.vector.tensor_scalar(out=ohP1[:, cs], in0=idxb[:],
                                            scalar1=iota[:, 1:2], scalar2=None,
                                            op0=AOP.is_equal)
                    idxf = bcp.tile([128, TCH], F32, tag="bcC")
                    nc.tensor.matmul(idxf[:], sel5[:, 128:256], data[:, cs],
                                     start=True, stop=True)
                    nc.vector.tensor_scalar(out=ohF[:, cs], in0=idxf[0:64, :],
                                            scalar1=iota[0:64, 0:1], scalar2=None,
                                            op0=AOP.is_equal)
                    pp = mmpA.tile([32, TCH], F32, tag="lnS")
                    nc.tensor.matmul(pp[:], embP[:, 0:32], ohP0[:, cs],
                                     start=True, stop=False)
                    nc.tensor.matmul(pp[:], embP[:, 32:64], ohP1[:, cs],
                                     start=False, stop=True)
                    nc.scalar.activation(c1[0:32, cs], pp[:], AF.Copy)
                    pl = mmpB.tile([32, TCH], F32, tag="lnS2")
                    nc.tensor.matmul(pl[:], lenW[:], data[:, cs],
                                     start=True, stop=True)
                    nc.scalar.activation(c1[32:64, cs], pl[:], AF.Identity,
                                         bias=lenB[:])
                    pf = mmpA.tile([32, TCH], F32, tag="lnS")
                    nc.tensor.matmul(pf[:], embF[:], ohF[:, cs],
                                     start=True, stop=True)
                    nc.scalar.activation(c1[64:96, cs], pf[:], AF.Copy)
                    pi = mmpB.tile([32, TCH], F32, tag="lnS2")
                    nc.tensor.matmul(pi[:], iatW[:], data[:, cs],
                                     start=True, stop=True)
                    nc.scalar.activation(c1[96:128, cs], pi[:], AF.Identity,
                                         bias=iatB[:])
                    pd = mmpA.tile([8, TCH], F32, tag="lnS")
                    nc.tensor.matmul(pd[:], dirD[:], data[:, cs],
                                     start=True, stop=True)
                    nc.scalar.activation(c2[:, cs], pd[:], AF.Identity, bias=dirB[:])
                # fus + LN -> feat
                for ch in range(NCH):
                    cs = slice(ch * TCH, (ch + 1) * TCH)
                    xln = []
                    for mblk in range(2):
                        ps = mmpC.tile([128, TCH], F32, tag="mmC")
                        nc.tensor.matmul(ps[:],
                                         fusW1[:, mblk * 128:(mblk + 1) * 128],
                                         rhs=c1[:, cs], start=True, stop=False)
                        nc.tensor.matmul(ps[:],
                                         fusW2[:, mblk * 128:(mblk + 1) * 128],
                                         rhs=c2[:, cs], start=False, stop=True)
                        xt = tmp2.tile([128, TCH], F32, tag="lnx")
                        nc.scalar.activation(xt[:], ps[:], AF.Identity,
                                             bias=fusB[:, mblk:mblk + 1])
                        xln.append(xt)
                    layer_norm_chunk(xln[0][:], xln[1][:],
                                     F[0][:, cs], F[1][:, cs], lnWB)

            # ================ mamba layers ================
            wpool2 = ctx.enter_context(tc.tile_pool(name="wbig", bufs=1))
            wpool1 = ctx.enter_context(tc.tile_pool(name="wsmall", bufs=1))
            apool = ctx.enter_context(tc.tile_pool(name="acts", bufs=1))
            spA = ctx.enter_context(tc.tile_pool(name="spA", bufs=2))
            spB = ctx.enter_context(tc.tile_pool(name="spB", bufs=1))
            spH = ctx.enter_context(tc.tile_pool(name="spH", bufs=1))

            for li in range(n_layers):
                W = {}
                for d in ("f", "r"):
                    pre = f"m{li}{d}_"
                    inW = wpool2.tile([128, 2048], F32, tag="inW" + d)
                    nc.sync.dma_start(out=inW[:, 0:1024],
                                      in_=P[pre + "inW"][0:128, :])
                    nc.sync.dma_start(out=inW[:, 1024:2048],
                                      in_=P[pre + "inW"][128:256, :])
                    outW = wpool1.tile([128, 1024], F32, tag="outW" + d)
                    for k in range(4):
                        nc.sync.dma_start(
                            out=outW[:, k * 256:(k + 1) * 256],
                            in_=P[pre + "outW"][k * 128:(k + 1) * 128, :])
                    xpW = wpool1.tile([128, 192], BF16, tag="xpW" + d)
                    for k in range(4):
                        nc.sync.dma_start(
                            out=xpW[:, k * 48:(k + 1) * 48],
                            in_=P[pre + "xpW"][k * 128:(k + 1) * 128, :])
                    dtW = wpool1.tile([16, 512], F32, tag="dtW" + d)
                    nc.sync.dma_start(out=dtW[:], in_=P[pre + "dtW"][:])
                    Asb = wpool1.tile([128, 64], F32, tag="A" + d)
                    nc.sync.dma_start(out=Asb[:], in_=P[pre + "A"][:])
                    convW = wpool1.tile([128, 16], F32, tag="convW" + d)
                    nc.sync.dma_start(out=convW[:], in_=P[pre + "convW"][:])
                    convB = wpool1.tile([128, 8], F32, tag="convB" + d)
                    nc.sync.dma_start(out=convB[:], in_=P[pre + "convB"][:])
                    dtB = wpool1.tile([128, 4], F32, tag="dtB" + d)
                    nc.sync.dma_start(out=dtB[:], in_=P[pre + "dtB"][:])
                    Dsb = wpool1.tile([128, 4], F32, tag="D" + d)
                    nc.sync.dma_start(out=Dsb[:], in_=P[pre + "D"][:])
                    W[d] = dict(inW=inW, outW=outW, xpW=xpW, dtW=dtW, A=Asb,
                                convW=convW, convB=convB, dtB=dtB, D=Dsb)

                for b in range(BLOC):
                    base = b * L
                    y2 = {}
                    for d in ("f", "r"):
                        rev = (d == "r")
                        Wd = W[d]
                        # ---- in_proj -> xi (raw), zs (silu(z))
                        xi = [apool.tile([128, L], BF16, tag=f"xi{k}", name=f"xi{k}")
                              for k in range(4)]
                        zs = [apool.tile([128, L], BF16, tag=f"zs{k}", name=f"zs{k}")
                              for k in range(4)]
                        for m in range(8):
                            for t2 in range(2):
                                cs = slice(base + t2 * TCH, base + (t2 + 1) * TCH)
                                ls = slice(t2 * TCH, (t2 + 1) * TCH)
                                ps = mmpC.tile([128, TCH], F32, tag="mmC")
                                nc.tensor.matmul(
                                    ps[:], Wd["inW"][:, m * 128:(m + 1) * 128],
                                    rhs=F[0][:, cs], start=True, stop=False)
                                nc.tensor.matmul(
                                    ps[:],
                                    Wd["inW"][:, 1024 + m * 128:1024 + (m + 1) * 128],
                                    rhs=F[1][:, cs], start=False, stop=True)
                                if m < 4:
                                    nc.scalar.activation(xi[m][:, ls], ps[:],
                                                         AF.Copy)
                                else:
                                    e_z = tmp2.tile([128, TCH], F32, tag="lnsq",
                                                    name="e_z")
                                    nc.scalar.activation(e_z[:], ps[:], AF.Exp,
                                                         scale=-1.0)
                                    sp_z = tmp2.tile([128, TCH], F32, tag="lnsq",
                                                     name="sp_z")
                                    nc.scalar.activation(sp_z[:], e_z[:], AF.Ln,
                                                         bias=1.0)
                                    sg_z = tmp2.tile([128, TCH], F32, tag="lnsq",
                                                     name="sg_z")
                                    nc.scalar.activation(sg_z[:], sp_z[:], AF.Exp,
                                                         scale=-1.0)
                                    nc.vector.tensor_tensor(
                                        out=zs[m - 4][:, ls], in0=ps[:],
                                        in1=sg_z[:], op=AOP.mult)
                        # ---- depthwise conv (causal fwd / anti-causal rev) + silu
                        xc = [apool.tile([128, L], BF16, tag=f"xi{k}",
                                          name=f"xc{k}") for k in range(4)]
                        for k4 in range(4):
                            cw = Wd["convW"]
                            acc = tmp2.tile([128, L], F32, tag="convacc")
                            nc.vector.tensor_scalar(
                                out=acc[:], in0=xi[k4][:],
                                scalar1=cw[:, k4 * 4 + 3:k4 * 4 + 4], scalar2=None,
                                op0=AOP.mult)
                            for k in range(3):
                                sh = 3 - k
                                if not rev:
                                    oap = acc[:, sh:L]
                                    iap = xi[k4][:, 0:L - sh]
                                else:
                                    oap = acc[:, 0:L - sh]
                                    iap = xi[k4][:, sh:L]
                                nc.vector.scalar_tensor_tensor(
                                    out=oap, in0=iap,
                                    scalar=cw[:, k4 * 4 + k:k4 * 4 + k + 1],
                                    in1=oap, op0=AOP.mult, op1=AOP.add)
                            e_x = tmp2.tile([128, L], F32, tag="sigx", name="e_x")
                            nc.scalar.activation(
                                e_x[:], acc[:], AF.Exp, scale=-1.0,
                                bias=Wd["convB"][:, 4 + k4:5 + k4])
                            sp_x = tmp2.tile([128, L], F32, tag="sigx", name="sp_x")
                            nc.scalar.activation(sp_x[:], e_x[:], AF.Ln, bias=1.0)
                            sg_x = tmp2.tile([128, L], F32, tag="sigx", name="sg_x")
                            nc.scalar.activation(sg_x[:], sp_x[:], AF.Exp,
                                                 scale=-1.0)
                            nc.vector.scalar_tensor_tensor(
                                out=xc[k4][:], in0=acc[:],
                                scalar=Wd["convB"][:, k4:k4 + 1], in1=sg_x[:],
                                op0=AOP.add, op1=AOP.mult)
                        # ---- xproj -> dtin [16, L], bc [32, L] (B | C)
                        dtin = tmp2.tile([16, L], F32, tag="convacc",
                                          name="dtin")
                        bc = apool.tile([32, L], BF16, tag="bcs")
                        for t2 in range(2):
                            ls = slice(t2 * TCH, (t2 + 1) * TCH)
                            ps = mmpC.tile([16, TCH], F32, tag="mmC")
                            for k in range(4):
                                nc.tensor.matmul(
                                    ps[:],
                                    Wd["xpW"][:, k * 48:k * 48 + 16],
                                    rhs=xc[k][:, ls], start=(k == 0),
                                    stop=(k == 3))
                            nc.scalar.activation(dtin[:, ls], ps[:], AF.Copy)
                            ps2 = mmpC.tile([32, TCH], F32, tag="mmC")
                            for k in range(4):
                                nc.tensor.matmul(
                                    ps2[:],
                                    Wd["xpW"][:, k * 48 + 16:(k + 1) * 48],
                                    rhs=xc[k][:, ls], start=(k == 0),
                                    stop=(k == 3))
                            nc.scalar.activation(bc[:, ls], ps2[:], AF.Copy)
                        # ---- dt = softplus(dt_w @ dt_in + dt_b)
                        dt = [apool.tile([128, L], F32, tag=f"dt{k}", name=f"dt{k}")
                              for k in range(4)]
                        for k4 in range(4):
                            for t2 in range(2):
                                ls = slice(t2 * TCH, (t2 + 1) * TCH)
                                ps = mmpC.tile([128, TCH], F32, tag="mmC")
                                nc.tensor.matmul(
                                    ps[:], Wd["dtW"][:, k4 * 128:(k4 + 1) * 128],
                                    rhs=dtin[:, ls], start=True, stop=True)
                                e_d = tmp2.tile([128, TCH], F32, tag="lnsq",
                                                name="e_d")
                                nc.scalar.activation(e_d[:], ps[:], AF.Exp,
                                                     bias=Wd["dtB"][:, k4:k4 + 1])
                                nc.scalar.activation(dt[k4][:, ls], e_d[:], AF.Ln,
                                                     bias=1.0)
                        # ---- w = dt * xc
                        wti = [apool.tile([128, L], BF16, tag=f"w{k}", name=f"w{k}")
                               for k in range(4)]
                        for k4 in range(4):
                            nc.vector.tensor_tensor(out=wti[k4][:], in0=dt[k4][:],
                                                    in1=xc[k4][:], op=AOP.mult)
                        # ---- state scan over n, accumulate y
                        yacc = [apool.tile([128, L], F32, tag=f"ya{k}",
                                            name=f"ya{k}", bufs=2)
                                for k in range(4)]
                        for n in range(NST):
                            Bb = bcp.tile([128, L], F32, tag="bcB")
                            Cb = bcp.tile([128, L], F32, tag="bcC")
                            for t2 in range(2):
                                ls = slice(t2 * TCH, (t2 + 1) * TCH)
                                nc.tensor.matmul(
                                    Bb[:, ls], selBC[:, n * 128:(n + 1) * 128],
                                    rhs=bc[:, ls], start=True, stop=True)
                                nc.tensor.matmul(
                                    Cb[:, ls],
                                    selBC[:, (16 + n) * 128:(17 + n) * 128],
                                    rhs=bc[:, ls], start=True, stop=True)
                            Bsb = apool.tile([128, L], BF16, tag="Bsb",
                                             name="Bsb", bufs=2)
                            nc.scalar.activation(Bsb[:], Bb[:], AF.Copy)
                            Csb = apool.tile([128, L], BF16, tag="Csb",
                                             name="Csb", bufs=2)
                            nc.scalar.activation(Csb[:], Cb[:], AF.Copy)
                            for k4 in range(4):
                                dA = spA.tile([128, L], F32, tag="dA")
                                din = dt[k4][:, ::-1] if rev else dt[k4][:]
                                nc.scalar.activation(
                                    dA[:], din, AF.Exp,
                                    scale=Wd["A"][:, k4 * 16 + n:k4 * 16 + n + 1])
                                dB = spB.tile([128, L], BF16, tag="dB")
                                if rev:
                                    nc.vector.tensor_tensor(
                                        out=dB[:], in0=wti[k4][:, ::-1],
                                        in1=Bsb[:, ::-1], op=AOP.mult)
                                else:
                                    nc.vector.tensor_tensor(
                                        out=dB[:], in0=wti[k4][:], in1=Bsb[:],
                                        op=AOP.mult)
                                h = spH.tile([128, L], BF16, tag="h")
                                nc.vector.tensor_tensor_scan(
                                    out=h[:], data0=dA[:], data1=dB[:],
                                    initial=0.0, op0=AOP.mult, op1=AOP.add)
                                hap = h[:, ::-1] if rev else h[:]
                                if n == 0:
                                    nc.vector.tensor_tensor(
                                        out=yacc[k4][:], in0=hap, in1=Csb[:],
                                        op=AOP.mult)
                                else:
                                    p = spA.tile([128, L], BF16, tag="dA",
                                                 name="p")
                                    nc.vector.tensor_tensor(
                                        out=p[:], in0=hap, in1=Csb[:], op=AOP.mult)
                                    nc.gpsimd.tensor_tensor(
                                        out=yacc[k4][:], in0=yacc[k4][:],
                                        in1=p[:], op=AOP.add)
                        # ---- tail: y2 = (yacc + xc*D) * silu(z)
                        for k4 in range(4):
                            nc.vector.scalar_tensor_tensor(
                                out=xc[k4][:], in0=xc[k4][:],
                                scalar=Wd["D"][:, k4:k4 + 1], in1=yacc[k4][:],
                                op0=AOP.mult, op1=AOP.add)
                            nc.vector.tensor_tensor(out=yacc[k4][:],
                                                    in0=xc[k4][:],
                                                    in1=zs[k4][:], op=AOP.mult)
                        y2[d] = yacc
                    # ---- fused out_proj (0.5 in weights) + residual + LN
                    for t2 in range(2):
                        cs = slice(base + t2 * TCH, base + (t2 + 1) * TCH)
                        ls = slice(t2 * TCH, (t2 + 1) * TCH)
                        xln = []
                        for mblk in range(2):
                            ps = mmpC.tile([128, TCH], F32, tag="mmC")
                            first = True
                            for d in ("f", "r"):
                                for k in range(4):
                                    nc.tensor.matmul(
                                        ps[:],
                                        W[d]["outW"][:, k * 256 + mblk * 128:
                                                     k * 256 + (mblk + 1) * 128],
                                        rhs=y2[d][k][:, ls], start=first,
                                        stop=(d == "r" and k == 3))
                                    first = False
                            xt = tmp2.tile([128, TCH], F32, tag="lnx")
                            nc.vector.tensor_tensor(out=xt[:], in0=ps[:],
                                                    in1=F[mblk][:, cs],
                                                    op=AOP.add)
                            xln.append(xt)
                        layer_norm_chunk(xln[0][:], xln[1][:],
                                         F[0][:, cs], F[1][:, cs], normWB)

            # ---------------- output
            nc.sync.dma_start(out=out_ext[0:128, :], in_=F[0][:])
            nc.sync.dma_start(out=out_ext[128:256, :], in_=F[1][:])
    return nc


# ---------------------------------------------------------------- host prep
def _blkfold(v):
    """[512] -> [128, 4] with col = dblk"""
    return np.ascontiguousarray(np.asarray(v, np.float32).reshape(4, 128).T)


def _pack_weights(tok, mambas_fwd, mambas_rev, norm_w, norm_b, n_layers=NL):
    import ml_dtypes
    w = {}
    misc = np.zeros((128, 12), np.float32)
    misc[:, 0] = np.arange(128, dtype=np.float32)
    misc[:, 1] = np.arange(128, 256, dtype=np.float32)
    misc[:, 2] = np.asarray(tok["ln_w"], np.float32)[0:128]
    misc[:, 3] = np.asarray(tok["ln_w"], np.float32)[128:256]
    misc[:, 4] = np.asarray(tok["ln_b"], np.float32)[0:128]
    misc[:, 5] = np.asarray(tok["ln_b"], np.float32)[128:256]
    misc[:, 6] = np.asarray(norm_w, np.float32)[0:128]
    misc[:, 7] = np.asarray(norm_w, np.float32)[128:256]
    misc[:, 8] = np.asarray(norm_b, np.float32)[0:128]
    misc[:, 9] = np.asarray(norm_b, np.float32)[128:256]
    misc[:, 10] = 1e-5
    misc[:, 11] = 1.0
    w["miscC"] = misc
    ep = np.asarray(tok["emb_proto"], np.float32)      # [256, 32]
    w["embP"] = np.ascontiguousarray(np.concatenate([ep[:128], ep[128:]], axis=1))
    w["embF"] = np.asarray(tok["emb_flags"], np.float32)
    ed = np.asarray(tok["emb_dir"], np.float32)
    dd = np.zeros((5, 8), np.float32); dd[4] = ed[1] - ed[0]
    w["dirD"] = dd
    w["dirB"] = np.ascontiguousarray(ed[0].reshape(8, 1))
    lw = np.zeros((5, 32), np.float32); lw[1] = np.asarray(tok["len_w"], np.float32)[:, 0]
    w["lenW"] = lw
    w["lenB"] = np.ascontiguousarray(np.asarray(tok["len_b"], np.float32).reshape(32, 1))
    iw = np.zeros((5, 32), np.float32); iw[3] = np.asarray(tok["iat_w"], np.float32)[:, 0]
    w["iatW"] = iw
    w["iatB"] = np.ascontiguousarray(np.asarray(tok["iat_b"], np.float32).reshape(32, 1))
    fw = np.asarray(tok["fus_w"], np.float32).T        # [136, 256]
    w["fusW1"] = np.ascontiguousarray(fw[0:128])
    w["fusW2"] = np.ascontiguousarray(fw[128:136])
    w["fusB"] = np.ascontiguousarray(
        np.asarray(tok["fus_b"], np.float32).reshape(2, 128).T)

    sel = np.zeros((32, 32 * 128), np.float32)
    for j in range(32):
        sel[j, j * 128:(j + 1) * 128] = 1.0
    w["selBC"] = sel.astype(ml_dtypes.bfloat16)
    s5 = np.zeros((5, 256), np.float32)
    s5[0, 0:128] = 1.0
    s5[2, 128:256] = 1.0
    w["sel5"] = s5

    for li in range(n_layers):
        for d, p in (("f", mambas_fwd[li]), ("r", mambas_rev[li])):
            pre = f"m{li}{d}_"
            w[pre + "inW"] = np.ascontiguousarray(np.asarray(p["in_w"], np.float32).T)
            cw = np.asarray(p["conv_w"], np.float32).reshape(512, 4)
            w[pre + "convW"] = np.ascontiguousarray(
                cw.reshape(4, 128, 4).transpose(1, 0, 2).reshape(128, 16))
            cb = _blkfold(p["conv_b"])
            w[pre + "convB"] = np.ascontiguousarray(
                np.concatenate([cb, -cb], axis=1))
            w[pre + "xpW"] = np.ascontiguousarray(
                np.asarray(p["xproj_w"], np.float32).T).astype(ml_dtypes.bfloat16)
            w[pre + "dtW"] = np.ascontiguousarray(np.asarray(p["dt_w"], np.float32).T)
            w[pre + "dtB"] = _blkfold(p["dt_b"])
            A = -np.exp(np.asarray(p["A_log"], np.float32))  # [512, 16]
            w[pre + "A"] = np.ascontiguousarray(
                A.reshape(4, 128, 16).transpose(1, 0, 2).reshape(128, 64))
            w[pre + "D"] = _blkfold(p["D"])
            w[pre + "outW"] = np.ascontiguousarray(
                np.asarray(p["out_w"], np.float32).T * 0.5)
    return w


def _core_data(x, c):
    xb = np.asarray(x, np.float32)[c * BLOC:(c + 1) * BLOC]  # [4, 1024, 5]
    flat = xb.reshape(COLS, 5)
    d = np.empty((5, COLS), np.float32)
    d[0] = np.clip(flat[:, 0], 0, 255)
    d[1] = flat[:, 1]
    d[2] = np.clip(flat[:, 2], 0, 63)
    d[3] = flat[:, 3]
    d[4] = np.clip(flat[:, 4], 0, 1)
    return d


_NC_CACHE = {}


def get_nc(n_layers=NL):
    if n_layers not in _NC_CACHE:
        _NC_CACHE[n_layers] = build_nc(n_layers)
    return _NC_CACHE[n_layers]


def kernel(x, tok, mambas_fwd, mambas_rev, norm_w, norm_b):
    n_layers = len(mambas_fwd)
    nc = get_nc(n_layers)
    w = _pack_weights(tok, mambas_fwd, mambas_rev, norm_w, norm_b, n_layers)
    in_maps = []
    for c in range(NCORES):
        m = dict(w)
        m["data"] = _core_data(x, c)
        in_maps.append(m)
    res = run_bass_kernel_spmd(nc, in_maps, core_ids=list(range(NCORES)))
    outs = []
    for c in range(NCORES):
        o = res.results[c]["out"]                      # [256, 4096]
        outs.append(o.reshape(DM, BLOC, L).transpose(1, 2, 0))
    return np.concatenate(outs, axis=0).astype(np.float32)
